# revision 3
# baseline (speedup 1.0000x reference)
"""Trainium2 Bass kernel for nn_CIFAR10Net (SpykeTorch-style spiking CNN).

Data-parallel over batch: 32 samples -> 8 NeuronCores x 4 samples.
Per (sample,timestep) image: conv7x7 -> fire(15) -> maxpool2 -> conv5x5
-> fire(10) -> maxpool3 -> conv5x5 -> pot3; winner-take-all class on host.

All convs run on the tensor engine as accumulating matmuls over kernel
offsets, with binary spike activations in bf16 (exact) and weights in a
bf16 hi+lo decomposition (2 matmuls/offset) for fp32-class precision.
Layout tricks:
  - conv1: ky packed into the contraction dim (126 = 18ch x 7ky)
  - conv2: 3 images interleaved row-wise ("strip") so a 2D access pattern
    covers (y,img,x); 90-channel contraction
  - conv3: (ky,channel-group) packed contraction (125 = 25ch x 5ky) via an
    on-chip replicated layout; kx as free-dim base shift
"""
import json
import sys

import numpy as np
import ml_dtypes

for _p in ('/opt/trn_rl_repo', '/root/.axon_site/_ro/trn_rl_repo'):
    if _p not in sys.path:
        sys.path.append(_p)

from concourse import bass, mybir
from concourse.tile import TileContext
from concourse.bass_utils import run_bass_kernel_spmd

F32 = mybir.dt.float32
BF16 = mybir.dt.bfloat16
MAX = mybir.AluOpType.max
IS_GT = mybir.AluOpType.is_gt

B, T = 32, 15
NCORES = 8
BPC = B // NCORES
NIMG = BPC * T
NTRI = NIMG // 3
THR1, THR2 = 15.0, 10.0


# ---------------------------------------------------------------------------
# Workaround for walrus "Too many sync wait commands": split multi-wait
# instructions into preceding single-wait NoOps on the same engine stream.
# ---------------------------------------------------------------------------
def _split_multi_waits(bir: dict) -> None:
    for fn in bir.get("functions", []):
        for bb in fn.get("blocks", []):
            out = []
            for inst in bb.get("instructions", []):
                si = inst.get("sync_info")
                if si:
                    waits = si.get("on_wait") or []
                    if len(waits) > 1:
                        eng = inst.get("engine")
                        for k, w in enumerate(waits[:-1]):
                            out.append({
                                "debug": inst.get("debug", 0),
                                "engine": eng, "ins": [], "outs": [],
                                "name": f"{inst['name']}-ws{k}",
                                "opcode": "NoOp",
                                "sync_info": {"on_update": [], "on_wait": [w]},
                            })
                        si["on_wait"] = [waits[-1]]
                out.append(inst)
            bb["instructions"] = out


def _patch_nc(nc):
    orig = nc.to_json_bytes

    def patched():
        d = json.loads(orig())
        _split_multi_waits(d)
        return json.dumps(d).encode()

    nc.to_json_bytes = patched
    return nc


# ---------------------------------------------------------------------------
# Kernel build (per-core program; same program on all 8 cores)
# ---------------------------------------------------------------------------
def _build_nc():
    nc = bass.Bass()
    x1_d = nc.declare_dram_parameter("x1", [NIMG, 126, 30 * 36], BF16, isOutput=False)
    w1h_d = nc.declare_dram_parameter("w1h", [7, 126, 90], BF16, isOutput=False)
    w1l_d = nc.declare_dram_parameter("w1l", [7, 126, 90], BF16, isOutput=False)
    w2h_d = nc.declare_dram_parameter("w2h", [4, 5, 125, 250], BF16, isOutput=False)
    w2l_d = nc.declare_dram_parameter("w2l", [4, 5, 125, 250], BF16, isOutput=False)
    w3h_d = nc.declare_dram_parameter("w3h", [10, 5, 125, 200], BF16, isOutput=False)
    pot3_d = nc.declare_dram_parameter("pot3", [NIMG, 200, 4, 4], F32, isOutput=True)

    with TileContext(nc) as tc:
        with tc.tile_pool(name="wpool", bufs=1) as wp, \
             tc.tile_pool(name="x3pool", bufs=1) as x3p, \
             tc.tile_pool(name="xrpool", bufs=1) as xrp, \
             tc.tile_pool(name="x1pool", bufs=3) as x1p, \
             tc.tile_pool(name="x2pool", bufs=3) as x2p, \
             tc.tile_pool(name="spool", bufs=3) as sp, \
             tc.tile_pool(name="o3pool", bufs=2) as o3p:

            w1hs = wp.tile([126, 7, 90], BF16)
            w1ls = wp.tile([126, 7, 90], BF16)
            w2hs = wp.tile([125, 4, 5, 250], BF16)
            w2ls = wp.tile([125, 4, 5, 250], BF16)
            w3hs = wp.tile([125, 10, 5, 200], BF16)
            nc.sync.dma_start(out=w1hs, in_=w1h_d[:, :, :].rearrange("k p m -> p k m"))
            nc.sync.dma_start(out=w1ls, in_=w1l_d[:, :, :].rearrange("k p m -> p k m"))
            for G in range(4):
                nc.sync.dma_start(out=w2hs[:, G, :, :], in_=w2h_d[G, :, :, :].rearrange("k p m -> p k m"))
                nc.sync.dma_start(out=w2ls[:, G, :, :], in_=w2l_d[G, :, :, :].rearrange("k p m -> p k m"))
            # DMA APs with >=3 free dims (4D total) fail on this HW path;
            # split the w3 transpose-load per group so each DMA is 3D.
            for g in range(10):
                nc.sync.dma_start(out=w3hs[:, g, :, :],
                                  in_=w3h_d[g, :, :, :].rearrange("k p m -> p k m"))

            # spk2 pooled+padded, std layout: [ch, img, 8, 8]
            x3a = x3p.tile([128, NIMG, 8, 8], BF16)
            x3b = x3p.tile([122, NIMG, 8, 8], BF16)
            nc.vector.memset(x3a, 0.0)
            nc.vector.memset(x3b, 0.0)
            # ky-cropped planes: ck[c, ky, img, (4 rows x 8 cols)]
            cka = x3p.tile([128, 5, NIMG, 32], BF16)
            ckb = x3p.tile([122, 5, NIMG, 32], BF16)
            # (ky,cl)-packed replicated layout for conv3
            xr = xrp.tile([125, 10, NIMG, 32], BF16)

            with tc.tile_pool(name="ps1", bufs=3, space="PSUM") as ps1, \
                 tc.tile_pool(name="ps2", bufs=4, space="PSUM") as ps2, \
                 tc.tile_pool(name="ps3", bufs=1, space="PSUM") as ps3:
                x2map = {}

                def loop_sched(n):
                    for t in range(n):
                        yield ('l1', t)
                        if t >= 1:
                            yield ('l2', t - 1)
                    yield ('l2', n - 1)

                for phase, tri in loop_sched(NTRI):
                  if phase == 'l1':
                    # x2strip: [90, (17 rows x 3 imgs), 17 cols], zero borders
                    x2 = x2p.tile([90, 51, 17], BF16)
                    nc.vector.memset(x2[:, 0:3, :], 0.0)
                    nc.vector.memset(x2[:, 48:51, :], 0.0)
                    nc.vector.memset(x2[:, 3:48, 0:17:16], 0.0)

                    for j in range(3):
                        img = tri * 3 + j
                        # ---- layer 1: conv 7x7 (ky packed in contraction) ----
                        x1 = x1p.tile([126, 30, 36], BF16)
                        # bulk input loads ride the ACT HWDGE ring, keeping the
                        # SP ring free for the latency-critical x2r replication
                        nc.scalar.dma_start(out=x1, in_=x1_d[img, :, :].rearrange("p (r c) -> p r c", r=30))
                        s1 = sp.tile([90, 30, 30], BF16)
                        for (r0, nr) in ((0, 16), (16, 14)):
                            pc = ps1.tile([90, 16, 30], F32)
                            pcv = pc[:, 0:nr, :]
                            for kx in range(7):
                                nc.tensor.matmul(pcv, w1hs[:, kx, :], x1[:, r0:r0 + nr, kx:kx + 30],
                                                 start=(kx == 0), stop=False)
                                nc.tensor.matmul(pcv, w1ls[:, kx, :], x1[:, r0:r0 + nr, kx:kx + 30],
                                                 start=False, stop=(kx == 6))
                            nc.vector.tensor_scalar(s1[:, r0:r0 + nr, :], pcv, THR1, None, IS_GT)
                        # pool 2x2 -> x2strip interior (rows a=3+3y+j, cols 1+x)
                        m1 = sp.tile([90, 30, 15], BF16)
                        nc.vector.tensor_tensor(m1, s1[:, :, 0::2], s1[:, :, 1::2], MAX)
                        nc.vector.tensor_tensor(x2[:, 3 + j:48:3, 1:16], m1[:, 0::2, :], m1[:, 1::2, :], MAX)

                    # replicate strip into (ky,cl)-packed layout (SBUF->SBUF)
                    x2r = x2p.tile([125, 4, 39, 17], BF16)
                    for G in range(4):
                        gc = 25 if G < 3 else 15
                        for ky in range(5):
                            nc.sync.dma_start(out=x2r[ky * gc:(ky + 1) * gc, G, :, :],
                                              in_=x2[25 * G:25 * G + gc, 3 * ky:3 * ky + 39, :])
                    x2map[tri] = x2r
                    continue

                  if phase == 'l2':
                    # ---- layer 2: conv 5x5, (ky,cl)-packed contraction ----
                    xin = x2map.pop(tri)
                    for oh, (f0, f1) in enumerate(((0, 128), (128, 250))):
                        nf = f1 - f0
                        p2 = ps2.tile([128, 39, 13], F32)
                        p2v = p2[0:nf, :, :]
                        nmm = 0
                        for G in range(4):
                            gc = 25 if G < 3 else 15
                            for kx in range(5):
                                mv = xin[0:5 * gc, G, :, kx:kx + 13]
                                nc.tensor.matmul(p2v, w2hs[0:5 * gc, G, kx, f0:f1], mv,
                                                 start=(nmm == 0), stop=False)
                                nmm += 1
                                nc.tensor.matmul(p2v, w2ls[0:5 * gc, G, kx, f0:f1], mv,
                                                 start=False, stop=(nmm == 20))
                        s2 = sp.tile([128, 39, 13], BF16)
                        s2v = s2[0:nf, :, :]
                        nc.vector.tensor_scalar(s2v, p2v, THR2, None, IS_GT)
                        # pool 3x3 stride 3: cols then rows ((y,img) interleaved)
                        m2 = sp.tile([128, 39, 4], BF16)
                        m2v = m2[0:nf, :, :]
                        nc.vector.tensor_tensor(m2v, s2v[:, :, 0:12:3], s2v[:, :, 1:12:3], MAX)
                        nc.vector.tensor_tensor(m2v, m2v, s2v[:, :, 2:13:3], MAX)
                        m2y = m2v.rearrange("p (y i) x -> p y i x", i=3)
                        r2 = sp.tile([128, 4, 3, 4], BF16)
                        r2v = r2[0:nf, :, :, :]
                        nc.vector.tensor_tensor(r2v, m2y[:, 0:12:3, :, :], m2y[:, 1:12:3, :, :], MAX)
                        nc.vector.tensor_tensor(r2v, r2v, m2y[:, 2:13:3, :, :], MAX)
                        dst = (x3a if oh == 0 else x3b)
                        dstv = dst[0:nf, 3 * tri:3 * tri + 3, 2:6, 2:6]
                        nc.vector.tensor_copy(dstv, r2v.rearrange("p y i x -> p i y x"))

                # ---- ky-crop x3std into ck (DVE strided in, contiguous out) ----
                for ky in range(5):
                    nc.vector.tensor_copy(
                        cka[:, ky, :, :].rearrange("p i (r c) -> p i r c", r=4),
                        x3a[:, :, ky:ky + 4, :])
                    nc.vector.tensor_copy(
                        ckb[:, ky, :, :].rearrange("p i (r c) -> p i r c", r=4),
                        x3b[:, :, ky:ky + 4, :])

                # ---- partition-remap into xr: coarse SBUF->SBUF DMAs ----
                for g in range(10):
                    c0, c1 = 25 * g, 25 * g + 25
                    for ky in range(5):
                        dv = xr[25 * ky:25 * ky + 25, g, :, :]
                        if c1 <= 128:
                            nc.sync.dma_start(out=dv, in_=cka[c0:c1, ky, :, :])
                        elif c0 >= 128:
                            nc.sync.dma_start(out=dv, in_=ckb[c0 - 128:c1 - 128, ky, :, :])
                        else:
                            na = 128 - c0
                            nc.sync.dma_start(out=dv[0:na, :, :], in_=cka[c0:128, ky, :, :])
                            nc.sync.dma_start(out=dv[na:25, :, :], in_=ckb[0:c1 - 128, ky, :, :])

                # ---- layer 3: conv 5x5, (ky,cl) contraction, kx via base shift ----
                xrv = xr[:, :, :, :].rearrange("p g i (r c) -> p g (i r) c", c=8)
                for q in range(4):
                    s0 = 60 * q
                    for oh, (f0, f1) in enumerate(((0, 128), (128, 200))):
                        nf = f1 - f0
                        p3 = ps3.tile([128, 60, 4], F32)
                        p3v = p3[0:nf, :, :]
                        n = 0
                        for g in range(10):
                            for kx in range(5):
                                mv = xrv[:, g, s0:s0 + 60, kx:kx + 4]
                                nc.tensor.matmul(p3v, w3hs[:, g, kx, f0:f1], mv,
                                                 start=(n == 0), stop=(n == 49))
                                n += 1
                        o3 = o3p.tile([128, 60, 4], F32)
                        o3v = o3[0:nf, :, :]
                        nc.vector.tensor_copy(o3v, p3v)
                        nc.scalar.dma_start(
                            out=pot3_d[15 * q:15 * q + 15, f0:f1, :, :].rearrange("b f y x -> f b y x"),
                            in_=o3v.rearrange("f (b y) x -> f b y x", y=4))
    _patch_nc(nc)
    return nc


_NC_CACHE = None


def _get_nc():
    global _NC_CACHE
    if _NC_CACHE is None:
        _NC_CACHE = _build_nc()
    return _NC_CACHE


# ---------------------------------------------------------------------------
# Host-side prep / post
# ---------------------------------------------------------------------------
def _prep(inp, w1, w2, w3):
    bf = ml_dtypes.bfloat16
    pad = np.zeros((B, T, 18, 36, 36), np.float32)
    pad[:, :, :, 2:34, 2:34] = inp
    x1 = np.empty((B, T, 126, 30, 36), np.float32)
    for ky in range(7):
        x1[:, :, 18 * ky:18 * ky + 18, :, :] = pad[:, :, :, ky:ky + 30, :]
    x1 = x1.reshape(B, T, 126, 30 * 36).astype(bf)

    def hilo(w):
        h = w.astype(bf)
        l = (w - h.astype(np.float32)).astype(bf)
        return np.asarray(h), np.asarray(l)

    w1h, w1l = hilo(np.ascontiguousarray(w1.transpose(3, 2, 1, 0).reshape(7, 126, 90)))
    w2t = np.zeros((4, 5, 125, 250), np.float32)
    for G in range(4):
        gc = 25 if G < 3 else 15
        blk = w2[:, 25 * G:25 * G + gc, :, :]
        w2t[G, :, 0:5 * gc, :] = blk.transpose(3, 2, 1, 0).reshape(5, 5 * gc, 250)
    w2h, w2l = hilo(w2t)
    w3t = np.empty((10, 5, 125, 200), np.float32)
    for g in range(10):
        blk = w3[:, 25 * g:25 * g + 25, :, :]
        w3t[g] = blk.transpose(3, 2, 1, 0).reshape(5, 125, 200)
    w3h, _ = hilo(w3t)
    return x1, (w1h, w1l, w2h, w2l, w3h)


def _winner_class(pot3):
    """Line-by-line port of the reference WTA (numpy, float32)."""
    t = pot3.shape[1]
    mask = (np.arange(t) == t - 1).astype(pot3.dtype)[None, :, None, None, None]
    spk3 = np.sign(pot3 * mask)
    b = pot3.shape[0]
    cnt = spk3.sum(axis=1)
    first = np.clip((t - cnt).astype(np.int32), 0, t - 1)
    vals = np.take_along_axis(pot3, first[:, None], axis=1)[:, 0]
    trunc = spk3 * vals[:, None]
    v = trunc.reshape(b, -1).max(axis=1) * t
    total = (trunc + spk3 * v[:, None, None, None, None]).sum(axis=1)
    flat = total.reshape(b, -1)
    idx = np.argmax(flat, axis=1)
    mx = np.max(flat, axis=1)
    hw = pot3.shape[-1] * pot3.shape[-2]
    feat = idx // hw
    return np.where(mx != 0, feat // 20, -1).astype(np.int32)


# ---------------------------------------------------------------------------
# numpy fallback (only used if max_layer != 3; the shipped config uses 3)
# ---------------------------------------------------------------------------
def _np_conv(x, w):
    b, t, c, h, wd = x.shape
    f, _, kh, kw = w.shape
    oh, ow = h - kh + 1, wd - kw + 1
    y = np.zeros((b, t, f, oh, ow), np.float32)
    for ky in range(kh):
        for kx in range(kw):
            patch = x[:, :, :, ky:ky + oh, kx:kx + ow]
            y += np.einsum('btchw,fc->btfhw', patch, w[:, :, ky, kx], optimize=True)
    return y


def _np_pool(x, k, s):
    b, t, c, h, w = x.shape
    oh, ow = (h - k) // s + 1, (w - k) // s + 1
    v = x[:, :, :, :oh * s, :ow * s].reshape(b, t, c, oh, s, ow, s)
    # max over the kxk window
    out = np.full((b, t, c, oh, ow), -np.inf, np.float32)
    for dy in range(k):
        for dx in range(k):
            out = np.maximum(out, x[:, :, :, dy:dy + oh * s:s, dx:dx + ow * s:s])
    return out


def _np_pad(x, p):
    return np.pad(x, ((0, 0), (0, 0), (0, 0), (p, p), (p, p)))


def _np_fire(pot, thr):
    th = np.where(pot > thr, pot, 0.0).astype(np.float32)
    return np.sign(th), th


def _numpy_reference(input, w1, w2, w3, max_layer):
    x = _np_pad(input.astype(np.float32), 2)
    spk1, pot1 = _np_fire(_np_conv(x, w1), THR1)
    if max_layer == 1:
        return spk1, pot1
    spk2, pot2 = _np_fire(_np_conv(_np_pad(_np_pool(spk1, 2, 2), 1), w2), THR2)
    if max_layer == 2:
        return spk2, pot2
    pot3 = _np_conv(_np_pad(_np_pool(spk2, 3, 3), 2), w3)
    t = pot3.shape[1]
    mask = (np.arange(t) == t - 1).astype(pot3.dtype)[None, :, None, None, None]
    spk3 = np.sign(pot3 * mask)
    cls = _winner_class(pot3)
    return cls, pot3


# ---------------------------------------------------------------------------
# Public entry point
# ---------------------------------------------------------------------------
def kernel(input, w1, w2, w3, max_layer):
    input = np.asarray(input, np.float32)
    w1 = np.asarray(w1, np.float32)
    w2 = np.asarray(w2, np.float32)
    w3 = np.asarray(w3, np.float32)
    ml = int(max_layer)
    if ml != 3 or input.shape != (B, T, 18, 32, 32):
        return _numpy_reference(input, w1, w2, w3, ml)

    x1, ws = _prep(input, w1, w2, w3)
    w1h, w1l, w2h, w2l, w3h = ws
    in_maps = []
    for c in range(NCORES):
        xc = x1[c * BPC:(c + 1) * BPC].reshape(NIMG, 126, 30 * 36)
        in_maps.append({"x1": np.ascontiguousarray(xc), "w1h": w1h, "w1l": w1l,
                        "w2h": w2h, "w2l": w2l, "w3h": w3h})
    nc = _get_nc()
    res = run_bass_kernel_spmd(nc, in_maps, list(range(NCORES)))
    pot3 = np.empty((B, T, 200, 4, 4), np.float32)
    for c in range(NCORES):
        pot3[c * BPC:(c + 1) * BPC] = res.results[c]["pot3"].reshape(BPC, T, 200, 4, 4)
    cls = _winner_class(pot3)
    return cls, pot3
